# revision 9
# baseline (speedup 1.0000x reference)
"""Trainium2 Bass kernel for nn_CrossAttention (B=4, S=1024, C=1024, H=16).

Sharding: 8 cores = (batch b in 0..4) x (query-half qs in 0..2). Each core
computes, for its 512 query rows of batch b: the Q projection, K/V
projections over the batch's *valid* key positions, masked-softmax
attention over all 16 heads, the output projection, and the MLP with
residual. No collectives.

Key structure (v2 of this kernel):
- The key mask is the same for every query row of a batch, so the host
  gathers only the valid key columns (mask OR over the two modalities) and
  zero-pads to SV (a multiple of 128, 896 for the reference mask). All
  K/QK/V/PV work shrinks from S=1024 to SV.
- Pad columns are written as exact zeros into kTa (the K eviction applies
  `(psum + bk) * kmask`), so their logits are 0 and exp(0)=1; the softmax
  denominator is corrected by subtracting the per-core pad count npad.
- Q/K biases are folded into the PSUM evictions (no beta/wstar matmul).
- Heads are packed in pairs on the 128 SBUF partitions (even head on
  partitions 0-63, odd head on 64-127). QK runs as two concurrent K=64
  matmuls on distinct PE row groups via tile_position (0,0)/(64,0), which
  the PE executes in parallel (~2x over one K=65 matmul).
- All activations flow transposed (contraction dim on partitions):
    qTa[:, m, :]  = Wq'^T qT + bq'   (pair m = heads 2m, 2m+1; ' = *SCALE)
    kTa[:, m, :]  = (Wk^T kT + bk) * kmask
    LT_h = kTa_h^T @ qTa_h           [kpos, q] per head, two heads at once
    PT_h = exp(LT_h)
    oT_h = [v_h | 1]^T @ PT_h        [65, q]; row 64 = denom + npad
    xT[h] = oT_h[0:64] * bcast(1/(denom_raw - npad))
    xpT = Wp^T xT + bp_eff           (bp_eff = bp + bv @ Wp, folded on host)
    h1T = gelu(W1^T xpT + b1)
    outT = xpT + W2^T h1T + b2
- Startup DMAs are issued round-robin across four engine queues so the
  first matmul isn't serialized behind one queue's descriptor stream.
"""

from contextlib import ExitStack

import numpy as np

import concourse.bass as bass
import concourse.tile as tile
from concourse import bacc, mybir
from concourse.bass_utils import run_bass_kernel_spmd

B, S, C, H = 4, 1024, 1024, 16
HD = C // H          # 64
SCALE = HD ** -0.5
P = 128              # SBUF partitions
SQ = S // 2          # 512 query rows per core
NCORES = 8
KT = C // P          # 8 contraction tiles of 128
NPAIR = H // 2       # 8 head pairs
N512 = 512
DENOM_EPS = 1e-20

F32 = mybir.dt.float32
BF16 = mybir.dt.bfloat16
NPBF16 = mybir.dt.np(BF16)


def build_program(SV):
    KTS = SV // P        # key-position tiles (7 for SV=896)
    NK = SV // 2         # K-proj eviction chunk width

    nc = bacc.Bacc(None, target_bir_lowering=False, debug=False)

    wq = nc.dram_tensor("wq", [C, C], BF16, kind="ExternalInput")
    wk = nc.dram_tensor("wk", [C, C], BF16, kind="ExternalInput")
    wv = nc.dram_tensor("wv", [C, C], BF16, kind="ExternalInput")
    wp = nc.dram_tensor("wp", [C, C], BF16, kind="ExternalInput")
    w1 = nc.dram_tensor("w1", [C, C], BF16, kind="ExternalInput")
    w2 = nc.dram_tensor("w2", [C, C], BF16, kind="ExternalInput")
    qt_in = nc.dram_tensor("qt_in", [C, SQ], BF16, kind="ExternalInput")
    kt_in = nc.dram_tensor("kt_in", [C, SV], BF16, kind="ExternalInput")
    vt_in = nc.dram_tensor("vt_in", [C, SV], BF16, kind="ExternalInput")
    kmask = nc.dram_tensor("kmask", [P, SV], BF16, kind="ExternalInput")
    # per-channel vectors packed to [P, 6, KT]:
    # i=0..4 -> bq', bk, bp_eff, b1, b2; [0, 5, 0] = npad
    bvecs = nc.dram_tensor("bvecs", [P, 6, KT], F32, kind="ExternalInput")
    out = nc.dram_tensor("out", [C, SQ], F32, kind="ExternalOutput")

    add = mybir.AluOpType.add
    mult = mybir.AluOpType.mult
    subtract = mybir.AluOpType.subtract
    Act = mybir.ActivationFunctionType

    with tile.TileContext(nc) as tc, ExitStack() as ctx:
        const = ctx.enter_context(tc.tile_pool(name="const", bufs=1))
        wfull = ctx.enter_context(tc.tile_pool(name="wfull", bufs=2))
        acts = ctx.enter_context(tc.tile_pool(name="acts", bufs=1))
        ptp = ctx.enter_context(tc.tile_pool(name="ptp", bufs=6))
        smal = ctx.enter_context(tc.tile_pool(name="smal", bufs=2))
        outp = ctx.enter_context(tc.tile_pool(name="outp", bufs=3))
        ps = ctx.enter_context(tc.tile_pool(name="ps", bufs=2, space="PSUM"))
        pslt = ctx.enter_context(tc.tile_pool(name="pslt", bufs=4, space="PSUM"))

        # ---- constants ----
        bv_sb = const.tile([P, 6, KT], F32, tag="bvecs")
        bq_sb = bv_sb[:, 0, :]
        bk_sb = bv_sb[:, 1, :]
        bp_sb = bv_sb[:, 2, :]
        b1_sb = bv_sb[:, 3, :]
        b2_sb = bv_sb[:, 4, :]
        npad_sb = bv_sb[0:1, 5, 0:1]
        km_sb = const.tile([P, SV], BF16, tag="kmask")

        # ---- input activations (chunk-loaded, resident) ----
        qin = acts.tile([P, KT, SQ], BF16, tag="qin_xT")
        kin = acts.tile([P, KT, SV], BF16, tag="kin_h1T")
        vin = acts.tile([P, KT, SV], BF16, tag="vin_xpT")

        # round-robin DMA issue across engines so no single queue
        # serializes the startup descriptor stream
        def chunked_load(dst, src, engines, nchunks=4):
            sr = src.rearrange("(k p) n -> p k n", p=P)
            step = KT // nchunks
            for i in range(nchunks):
                ks = slice(i * step, (i + 1) * step)
                engines[i % len(engines)].dma_start(dst[:, ks, :], sr[:, ks, :])

        # ---- intermediates, resident ----
        # head-pair packed: partitions 0-63 = even head, 64-127 = odd head
        qTa = acts.tile([P, NPAIR, SQ], BF16, tag="qTa")
        kTa = acts.tile([P, NPAIR, SV], BF16, tag="kTa")
        vaug = acts.tile([P, KTS, H * 65], BF16, tag="va")  # [kpos, h*(64|1)]
        xT = acts.tile([P, KT, SQ], BF16, tag="qin_xT")        # [c, q] attn out
        xpT = acts.tile([P, KT, SQ], BF16, tag="vin_xpT")      # [c', q] proj out
        h1T = acts.tile([P, KT, SQ], BF16, tag="kin_h1T")      # [c_h, q] hidden

        vaug_h = vaug.rearrange("p k (h e) -> p k h e", e=65)
        for kt in range(KTS):
            nc.vector.memset(vaug_h[:, kt, :, 64:65], 1.0)

        def load_w(w_dram, engines, nchunks=4):
            wsb = wfull.tile([P, KT, C], BF16, tag="w")
            sr = w_dram.rearrange("(k p) n -> p k n", p=P)
            step = KT // nchunks
            for i in range(nchunks):
                ks = slice(i * step, (i + 1) * step)
                engines[i % len(engines)].dma_start(wsb[:, ks, :], sr[:, ks, :])
            return wsb

        # ---- QK work queue: units of (head-pair, kpos tile), each emitting
        # two concurrent row-group matmuls + two exp ACTs. Drained a few
        # units at a time between other PE work so the ScalarEngine's exp
        # stream paces evenly and lt psum reuse never stalls the PE. ----
        pTts = {}
        qk_tasks = []

        def enqueue_qk(hm):
            pTtA = ptp.tile([P, KTS, N512], BF16, tag="pt")
            pTtB = ptp.tile([P, KTS, N512], BF16, tag="pt")
            pTts[2 * hm] = pTtA
            pTts[2 * hm + 1] = pTtB
            for kt in range(KTS):
                qk_tasks.append((hm, kt))

        def drain_qk(n):
            for _ in range(min(n, len(qk_tasks))):
                hm, kt = qk_tasks.pop(0)
                ltA = pslt.tile([P, N512], F32, tag="lt")
                ltB = pslt.tile([P, N512], F32, tag="lt")
                nc.tensor.matmul(
                    ltA[:, :],
                    kTa[0:HD, hm, kt * P:(kt + 1) * P],
                    qTa[0:HD, hm, :],
                    start=True, stop=True, tile_position=(0, 0),
                )
                nc.tensor.matmul(
                    ltB[:, :],
                    kTa[HD:P, hm, kt * P:(kt + 1) * P],
                    qTa[HD:P, hm, :],
                    start=True, stop=True, tile_position=(HD, 0),
                )
                nc.scalar.activation(
                    out=pTts[2 * hm][:, kt, :], in_=ltA[:, :], func=Act.Exp)
                nc.scalar.activation(
                    out=pTts[2 * hm + 1][:, kt, :], in_=ltB[:, :], func=Act.Exp)

        # ---- Q projection (transposed output, bias folded into eviction) ----
        chunked_load(qin, qt_in, [nc.sync, nc.gpsimd])
        wsb = load_w(wq, [nc.scalar, nc.sync])
        nc.sync.dma_start(bv_sb[:, :, :], bvecs[:, :, :])
        nc.gpsimd.dma_start(km_sb[:, :], kmask[:, :])
        chunked_load(kin, kt_in, [nc.gpsimd, nc.sync])
        for m in range(KT):
            pt = ps.tile([P, N512], F32, tag="mm")
            for k in range(KT):
                nc.tensor.matmul(
                    pt[:, :],
                    wsb[:, k, m * P:(m + 1) * P],
                    qin[:, k, :],
                    start=(k == 0), stop=(k == KT - 1),
                )
            nc.vector.tensor_scalar(
                out=qTa[:, m, :], in0=pt[:, :],
                scalar1=bq_sb[:, m:m + 1], scalar2=None, op0=add,
            )

        # ---- K projection; QK pairs start as soon as their kTa tile lands ----
        wsb = load_w(wk, [nc.scalar, nc.gpsimd])
        for m in range(KT):
            for n in range(2):
                pt = ps.tile([P, NK], F32, tag="mm")
                for k in range(KT):
                    nc.tensor.matmul(
                        pt[:, :],
                        wsb[:, k, m * P:(m + 1) * P],
                        kin[:, k, n * NK:(n + 1) * NK],
                        start=(k == 0), stop=(k == KT - 1),
                    )
                ns = slice(n * NK, (n + 1) * NK)
                nc.vector.scalar_tensor_tensor(
                    out=kTa[:, m, ns], in0=pt[:, :],
                    scalar=bk_sb[:, m:m + 1], in1=km_sb[:, ns],
                    op0=add, op1=mult,
                )
            enqueue_qk(m)
            drain_qk(2)
        chunked_load(vin, vt_in, [nc.sync, nc.gpsimd])

        # ---- attention: V projection + remaining QK + PV pipeline ----
        wsb = load_w(wv, [nc.gpsimd, nc.sync])

        def emit_v_chunk(i):
            # kpos tile m, c_out chunk n
            m, n = i % KTS, i // KTS
            pt = ps.tile([P, N512], F32, tag="mm")
            for k in range(KT):
                nc.tensor.matmul(
                    pt[:, :],
                    vin[:, k, m * P:(m + 1) * P],
                    wsb[:, k, n * N512:(n + 1) * N512],
                    start=(k == 0), stop=(k == KT - 1),
                )
            nc.vector.tensor_copy(
                vaug_h[:, m, 8 * n:8 * n + 8, 0:64],
                pt[:, :].rearrange("p (h d) -> p h d", d=HD),
            )

        def emit_pv(h):
            hp = (h % 2) * HD
            hm = h // 2
            pTt = pTts.pop(h)
            pv = ps.tile([HD + 1, N512], F32, tag="mm")
            for kt in range(KTS):
                nc.tensor.matmul(
                    pv[:, :],
                    vaug[:, kt, h * 65:(h + 1) * 65],
                    pTt[:, kt, :],
                    start=(kt == 0), stop=(kt == KTS - 1),
                )
            rc = smal.tile([1, N512], F32, tag="rc")
            bc = smal.tile([HD, N512], F32, tag="bc")
            # denom_raw = true_denom + npad (pad cols give exp(0)=1);
            # true_denom >= nvalid * exp(min logit) >> 0, so the fast
            # reciprocal's denorm/zero edge cases cannot occur
            nc.vector.tensor_scalar(
                out=rc[0:1, :], in0=pv[HD:HD + 1, :],
                scalar1=npad_sb, scalar2=None, op0=subtract,
            )
            nc.vector.reciprocal_approx_fast(out=rc[0:1, :], in_=rc[0:1, :])
            nc.gpsimd.partition_broadcast(bc[:, :], rc[0:1, :])
            nc.vector.tensor_mul(xT[hp:hp + HD, hm, :], pv[0:HD, :], bc[:, :])

        # V chunks n=0 first (PV_0..7 read the full n=0 group of vaug),
        # then the PV pipeline with the remaining QKs and V chunks
        # interleaved.
        for i in range(KTS):
            emit_v_chunk(i)
            drain_qk(3)
        for h in range(H):
            emit_pv(h)
            drain_qk(3)
            if h < KTS:
                emit_v_chunk(KTS + h)
                drain_qk(2)
        drain_qk(len(qk_tasks))

        # ---- output projection + MLP ----
        wsb = load_w(wp, [nc.gpsimd, nc.sync])
        for m in range(KT):
            pt = ps.tile([P, N512], F32, tag="mm")
            for k in range(KT):
                nc.tensor.matmul(
                    pt[:, :], wsb[:, k, m * P:(m + 1) * P], xT[:, k, :],
                    start=(k == 0), stop=(k == KT - 1),
                )
            nc.vector.tensor_scalar(
                out=xpT[:, m, :], in0=pt[:, :],
                scalar1=bp_sb[:, m:m + 1], scalar2=None, op0=add,
            )

        wsb = load_w(w1, [nc.gpsimd, nc.sync])
        for m in range(KT):
            pt = ps.tile([P, N512], F32, tag="mm")
            for k in range(KT):
                nc.tensor.matmul(
                    pt[:, :], wsb[:, k, m * P:(m + 1) * P], xpT[:, k, :],
                    start=(k == 0), stop=(k == KT - 1),
                )
            nc.scalar.activation(
                out=h1T[:, m, :], in_=pt[:, :], func=Act.Gelu,
                bias=b1_sb[:, m:m + 1], scale=1.0,
            )

        wsb = load_w(w2, [nc.gpsimd, nc.sync])
        for m in range(KT):
            pt = ps.tile([P, N512], F32, tag="mm")
            for k in range(KT):
                nc.tensor.matmul(
                    pt[:, :], wsb[:, k, m * P:(m + 1) * P], h1T[:, k, :],
                    start=(k == 0), stop=(k == KT - 1),
                )
            ot = outp.tile([P, N512], F32, tag="o")
            nc.vector.scalar_tensor_tensor(
                out=ot[:, :], in0=pt[:, :], scalar=b2_sb[:, m:m + 1],
                in1=xpT[:, m, :], op0=add, op1=add,
            )
            nc.sync.dma_start(out[m * P:(m + 1) * P, :], ot[:, :])

    nc.compile()
    return nc


_prog_cache = {}


def _get_program(SV):
    if SV not in _prog_cache:
        _prog_cache[SV] = build_program(SV)
    return _prog_cache[SV]


def make_in_maps(inputs, SV, valid_idx, nvalid):
    q = np.asarray(inputs["query"], np.float32)
    k = np.asarray(inputs["key"], np.float32)
    v = np.asarray(inputs["value"], np.float32)
    Wq = np.asarray(inputs["Wq"], np.float32) * SCALE
    bq = np.asarray(inputs["bq"], np.float32) * SCALE
    Wk = np.asarray(inputs["Wk"], np.float32)
    bk = np.asarray(inputs["bk"], np.float32)
    Wv = np.asarray(inputs["Wv"], np.float32)
    bv = np.asarray(inputs["bv"], np.float32)
    Wp = np.asarray(inputs["Wp"], np.float32)
    bp = np.asarray(inputs["bp"], np.float32)
    W1 = np.asarray(inputs["W1"], np.float32)
    b1 = np.asarray(inputs["b1"], np.float32)
    W2 = np.asarray(inputs["W2"], np.float32)
    b2 = np.asarray(inputs["b2"], np.float32)

    bp_eff = bp + bv @ Wp

    shared = {
        "wq": np.ascontiguousarray(Wq.astype(NPBF16)),
        "wk": np.ascontiguousarray(Wk.astype(NPBF16)),
        "wv": np.ascontiguousarray(Wv.astype(NPBF16)),
        "wp": np.ascontiguousarray(Wp.astype(NPBF16)),
        "w1": np.ascontiguousarray(W1.astype(NPBF16)),
        "w2": np.ascontiguousarray(W2.astype(NPBF16)),
    }

    def pack_cols(vec):      # [C] -> [P, KT] with [p, j] = vec[j*128+p]
        return np.asarray(vec, np.float32).reshape(KT, P).T

    in_maps = []
    for core in range(NCORES):
        b, qs = divmod(core, 2)
        nv = int(nvalid[b])
        idx = valid_idx[b]
        m = dict(shared)
        m["qt_in"] = np.ascontiguousarray(
            q[b, qs * SQ:(qs + 1) * SQ, :].T.astype(NPBF16))
        ktg = np.zeros((C, SV), NPBF16)
        ktg[:, :nv] = k[b].T[:, idx].astype(NPBF16)
        vtg = np.zeros((C, SV), NPBF16)
        vtg[:, :nv] = v[b].T[:, idx].astype(NPBF16)
        m["kt_in"] = np.ascontiguousarray(ktg)
        m["vt_in"] = np.ascontiguousarray(vtg)
        kmrow = np.zeros((SV,), NPBF16)
        kmrow[:nv] = 1.0
        m["kmask"] = np.ascontiguousarray(np.broadcast_to(kmrow, (P, SV)))
        base = np.zeros((P, 6, KT), np.float32)
        for i, vec in enumerate((bq, bk, bp_eff, b1, b2)):
            base[:, i, :] = pack_cols(vec)
        base[0, 5, 0] = float(SV - nv)
        m["bvecs"] = np.ascontiguousarray(base)
        in_maps.append(m)
    return in_maps


def run(inputs, trace=False, trace_cores=None):
    mask = np.asarray(inputs["mask"])
    combined = (mask[:, :S] != 0) | (mask[:, S:2 * S] != 0)   # [B, S]
    valid_idx = [np.nonzero(combined[b])[0] for b in range(B)]
    nvalid = np.array([len(ix) for ix in valid_idx])
    SV = max(P, int(-(-int(nvalid.max()) // P)) * P)
    nc = _get_program(SV)
    in_maps = make_in_maps(inputs, SV, valid_idx, nvalid)
    res = run_bass_kernel_spmd(
        nc, in_maps, core_ids=list(range(NCORES)),
        trace=trace, trace_cores=trace_cores,
    )
    outfull = np.empty((B, S, C), np.float32)
    for core in range(NCORES):
        b, qs = divmod(core, 2)
        outfull[b, qs * SQ:(qs + 1) * SQ, :] = res.results[core]["out"].T
    return outfull, res


def kernel(**inputs):
    outfull, _ = run(inputs)
    return outfull


# revision 10
# speedup vs baseline: 1.0816x; 1.0816x over previous
"""Trainium2 Bass kernel for nn_CrossAttention (B=4, S=1024, C=1024, H=16).

Sharding: 8 cores = (batch b in 0..4) x (query-half qs in 0..2). Each core
computes, for its 512 query rows of batch b: the Q projection, K/V
projections over the batch's *valid* key positions, masked-softmax
attention over all 16 heads, the output projection, and the MLP with
residual. No collectives.

Key structure (v3 of this kernel):
- The key mask is identical for every query row of a batch, so the host
  gathers only the valid key columns (mask OR over the two modalities) and
  zero-pads to SV (a multiple of 128; 896 for the reference mask). All
  K/QK/V/PV work shrinks from S=1024 to SV.
- Pad columns are written as exact zeros into kTa (the K eviction applies
  `(psum + bk) * kmask`), so their logits are 0 and exp(0)=1; the softmax
  denominator is corrected by subtracting the per-core pad count npad.
- Q/K biases are folded into the PSUM evictions (no beta matmul).
- QK matmuls use a 65-row contraction (64 head dims + one zeroed dummy
  row) so the PE stays in the full 128x128 tiling mode; K=64 operands
  would switch the array into 64-row tiling mode and pay a drain on every
  transition with the surrounding 128-row matmuls.
- The MLP matmuls run in fp8(e4m3) DoubleRow mode (2 contraction rows per
  PE cell, ~1.7x): host ships W1/W2 pre-scaled by 2^12; the P-projection
  eviction writes both a bf16 residual copy (with bp_eff+b2) and a scaled
  fp8 copy; gelu un-scales via the ACT scale; the final eviction un-scales
  the W2 product while adding the residual.
- All activations flow transposed (contraction dim on SBUF partitions):
    qTa[0:64, h, :]  = Wq'^T qT + bq'     (' = *SCALE)
    kTa[0:64, h, :]  = (Wk^T kT + bk) * kmask
    LT_h = kTa_h^T @ qTa_h                [kpos, q]
    PT_h = exp(LT_h)
    oT_h = [v_h | 1]^T @ PT_h             [65, q]; row 64 = denom + npad
    xT[h] = oT_h[0:64] * bcast(1/(denom_raw - npad))
    xpT   = Wp^T xT + bp_eff (+b2 in the residual copy)
    h1T   = gelu((W1s^T xp8) * inv1 + b1)
    outT  = xpT_res + (W2s^T h18) * inv2
- Startup DMAs get dedicated queues (weights m-chunked on the scalar
  queue so the first output tile's weights land first) instead of
  serializing ~27 descriptors behind one queue.
"""

from contextlib import ExitStack

import numpy as np

import concourse.bass as bass
import concourse.tile as tile
from concourse import bacc, mybir
from concourse.bass_utils import run_bass_kernel_spmd

B, S, C, H = 4, 1024, 1024, 16
HD = C // H          # 64
SCALE = HD ** -0.5
P = 128              # SBUF partitions
SQ = S // 2          # 512 query rows per core
NCORES = 8
KT = C // P          # 8 contraction tiles of 128
N512 = 512

SXP = 1024.0         # fp8 scale for xp (M1 input)
SH = 2048.0          # fp8 scale for h1 (M2 input)
SW = 4096.0          # fp8 scale for W1/W2
INV1 = 1.0 / (SXP * SW)
INV2 = 1.0 / (SH * SW)

F32 = mybir.dt.float32
BF16 = mybir.dt.bfloat16
FP8 = mybir.dt.float8e4
NPBF16 = mybir.dt.np(BF16)
NPFP8 = mybir.dt.np(FP8)
FP8MAX = 240.0


def build_program(SV):
    KTS = SV // P        # key-position tiles (7 for SV=896)
    NK = SV // 2         # K-proj eviction chunk width

    nc = bacc.Bacc(None, target_bir_lowering=False, debug=False)

    wq = nc.dram_tensor("wq", [C, C], BF16, kind="ExternalInput")
    wk = nc.dram_tensor("wk", [C, C], BF16, kind="ExternalInput")
    wv = nc.dram_tensor("wv", [C, C], BF16, kind="ExternalInput")
    wp = nc.dram_tensor("wp", [C, C], BF16, kind="ExternalInput")
    w1 = nc.dram_tensor("w1", [C, C], FP8, kind="ExternalInput")
    w2 = nc.dram_tensor("w2", [C, C], FP8, kind="ExternalInput")
    qt_in = nc.dram_tensor("qt_in", [C, SQ], BF16, kind="ExternalInput")
    kt_in = nc.dram_tensor("kt_in", [C, SV], BF16, kind="ExternalInput")
    vt_in = nc.dram_tensor("vt_in", [C, SV], BF16, kind="ExternalInput")
    kmask = nc.dram_tensor("kmask", [P, SV], BF16, kind="ExternalInput")
    # per-channel vectors packed to [P, 6, KT]:
    # i=0..4 -> bq', bk, bp_eff, b1, bp_eff+b2; [0, 5, 0] = npad
    bvecs = nc.dram_tensor("bvecs", [P, 6, KT], F32, kind="ExternalInput")
    out = nc.dram_tensor("out", [C, SQ], F32, kind="ExternalOutput")

    add = mybir.AluOpType.add
    mult = mybir.AluOpType.mult
    subtract = mybir.AluOpType.subtract
    Act = mybir.ActivationFunctionType
    DR = mybir.MatmulPerfMode.DoubleRow

    with tile.TileContext(nc) as tc, ExitStack() as ctx:
        const = ctx.enter_context(tc.tile_pool(name="const", bufs=1))
        wfull = ctx.enter_context(tc.tile_pool(name="wfull", bufs=2))
        acts = ctx.enter_context(tc.tile_pool(name="acts", bufs=1))
        ptp = ctx.enter_context(tc.tile_pool(name="ptp", bufs=5))
        smal = ctx.enter_context(tc.tile_pool(name="smal", bufs=2))
        outp = ctx.enter_context(tc.tile_pool(name="outp", bufs=3))
        ps = ctx.enter_context(tc.tile_pool(name="ps", bufs=2, space="PSUM"))
        pslt = ctx.enter_context(tc.tile_pool(name="pslt", bufs=3, space="PSUM"))

        # ---- constants ----
        bv_sb = const.tile([P, 6, KT], F32, tag="bvecs")
        bq_sb = bv_sb[:, 0, :]
        bk_sb = bv_sb[:, 1, :]
        bp_sb = bv_sb[:, 2, :]
        b1_sb = bv_sb[:, 3, :]
        bpb2_sb = bv_sb[:, 4, :]
        npad_sb = bv_sb[0:1, 5, 0:1]
        km_sb = const.tile([P, SV], BF16, tag="kmask")

        # ---- input activations (chunk-loaded, resident) ----
        qin = acts.tile([P, KT, SQ], BF16, tag="qin_xT")
        kin = acts.tile([P, KT, SV], BF16, tag="kin_h1T")
        vin = acts.tile([P, KT, SV], BF16, tag="vin_xpT")

        def chunked_load(dst, src, engines, nchunks=4):
            sr = src.rearrange("(k p) n -> p k n", p=P)
            step = KT // nchunks
            for i in range(nchunks):
                ks = slice(i * step, (i + 1) * step)
                engines[i % len(engines)].dma_start(dst[:, ks, :], sr[:, ks, :])

        # weights chunked along output columns (m), so the first output
        # tile's weights arrive first and the GEMM can start immediately
        def load_w_mchunks(w_dram, engine, dtype=BF16, tag="w", nchunks=4):
            wsb = wfull.tile([P, KT, C], dtype, tag=tag)
            sr = w_dram.rearrange("(k p) n -> p k n", p=P)
            step = C // nchunks
            for i in range(nchunks):
                ns = slice(i * step, (i + 1) * step)
                engine.dma_start(wsb[:, :, ns], sr[:, :, ns])
            return wsb

        def load_w(w_dram, engines, dtype=BF16, tag="w", nchunks=4):
            wsb = wfull.tile([P, KT, C], dtype, tag=tag)
            sr = w_dram.rearrange("(k p) n -> p k n", p=P)
            step = KT // nchunks
            for i in range(nchunks):
                ks = slice(i * step, (i + 1) * step)
                engines[i % len(engines)].dma_start(wsb[:, ks, :], sr[:, ks, :])
            return wsb

        # ---- intermediates, resident ----
        # qTa/kTa: rows 0-63 head data, row 64 zeroed (keeps the QK
        # contraction at 65 rows -> full 128x128 PE tiling mode)
        qTa = acts.tile([P, H, SQ], BF16, tag="qTa")
        kTa = acts.tile([P, H, SV], BF16, tag="kTa")
        vaug = acts.tile([P, KTS, H * 65], BF16, tag="va")  # [kpos, h*(64|1)]
        xT = acts.tile([P, KT, SQ], BF16, tag="qin_xT")        # [c, q] attn out
        xpT = acts.tile([P, KT, SQ], BF16, tag="vin_xpT")      # [c', q] resid
        h1T = acts.tile([P, KT, SQ], BF16, tag="kin_h1T")      # [c_h, q] hidden
        xp8 = acts.tile([P, KT, SQ], FP8, tag="xp8")           # scaled M1 input
        h18 = acts.tile([P, KT, SQ], FP8, tag="h18")           # scaled M2 input

        nc.vector.memset(qTa[HD:HD + 1, :, :], 0.0)
        nc.vector.memset(kTa[HD:HD + 1, :, :], 0.0)
        vaug_h = vaug.rearrange("p k (h e) -> p k h e", e=65)
        for kt in range(KTS):
            nc.vector.memset(vaug_h[:, kt, :, 64:65], 1.0)

        # ---- QK work queue: units of (head, ktile-pair). Each unit is 1-2
        # matmuls into one [P, 2*N512] psum tile plus one exp ACT covering
        # both ktiles. Drained a few units at a time between other PE work
        # so the ScalarEngine's exp stream paces evenly. ----
        pTts = {}
        qk_tasks = []
        NUNIT = (KTS + 1) // 2

        def enqueue_qk(h):
            pTt = ptp.tile([P, KTS, N512], BF16, tag="pt")
            pTts[h] = pTt
            for u in range(NUNIT):
                qk_tasks.append((h, u))

        def drain_qk(n):
            for _ in range(min(n, len(qk_tasks))):
                h, u = qk_tasks.pop(0)
                kts = list(range(2 * u, min(2 * u + 2, KTS)))
                lt = pslt.tile([P, 2 * N512], F32, tag="lt")
                for j, kt in enumerate(kts):
                    nc.tensor.matmul(
                        lt[:, j * N512:(j + 1) * N512],
                        kTa[0:HD + 1, h, kt * P:(kt + 1) * P],
                        qTa[0:HD + 1, h, :],
                        start=True, stop=True,
                    )
                nj = len(kts)
                nc.scalar.activation(
                    out=pTts[h][:, 2 * u:2 * u + nj, :],
                    in_=lt[:, 0:nj * N512].rearrange("p (t n) -> p t n", n=N512),
                    func=Act.Exp,
                )

        # ---- Q projection (transposed output, bias folded into eviction) ----
        chunked_load(qin, qt_in, [nc.sync, nc.gpsimd])
        wsb = load_w_mchunks(wq, nc.scalar)
        nc.sync.dma_start(bv_sb[:, :, :], bvecs[:, :, :])
        nc.gpsimd.dma_start(km_sb[:, :], kmask[:, :])
        chunked_load(kin, kt_in, [nc.gpsimd, nc.sync])
        for m in range(KT):
            pt = ps.tile([P, N512], F32, tag="mm")
            for k in range(KT):
                nc.tensor.matmul(
                    pt[:, :],
                    wsb[:, k, m * P:(m + 1) * P],
                    qin[:, k, :],
                    start=(k == 0), stop=(k == KT - 1),
                )
            for j in range(2):
                nc.vector.tensor_scalar(
                    out=qTa[0:HD, 2 * m + j, :], in0=pt[j * HD:(j + 1) * HD, :],
                    scalar1=bq_sb[j * HD:(j + 1) * HD, m:m + 1],
                    scalar2=None, op0=add,
                )

        # ---- K projection; QK starts as soon as a head's kTa lands ----
        wsb = load_w_mchunks(wk, nc.scalar)
        for m in range(KT):
            for n in range(2):
                pt = ps.tile([P, NK], F32, tag="mm")
                for k in range(KT):
                    nc.tensor.matmul(
                        pt[:, :],
                        wsb[:, k, m * P:(m + 1) * P],
                        kin[:, k, n * NK:(n + 1) * NK],
                        start=(k == 0), stop=(k == KT - 1),
                    )
                ns = slice(n * NK, (n + 1) * NK)
                for j in range(2):
                    nc.vector.scalar_tensor_tensor(
                        out=kTa[0:HD, 2 * m + j, ns],
                        in0=pt[j * HD:(j + 1) * HD, :],
                        scalar=bk_sb[j * HD:(j + 1) * HD, m:m + 1],
                        in1=km_sb[j * HD:(j + 1) * HD, ns],
                        op0=add, op1=mult,
                    )
            enqueue_qk(2 * m)
            enqueue_qk(2 * m + 1)
            drain_qk(2)
        chunked_load(vin, vt_in, [nc.gpsimd, nc.sync])

        # ---- attention: V projection + remaining QK + PV pipeline ----
        wsb = load_w(wv, [nc.gpsimd, nc.sync])

        def emit_v_chunk(i):
            # kpos tile m, c_out chunk n
            m, n = i % KTS, i // KTS
            pt = ps.tile([P, N512], F32, tag="mm")
            for k in range(KT):
                nc.tensor.matmul(
                    pt[:, :],
                    vin[:, k, m * P:(m + 1) * P],
                    wsb[:, k, n * N512:(n + 1) * N512],
                    start=(k == 0), stop=(k == KT - 1),
                )
            nc.vector.tensor_copy(
                vaug_h[:, m, 8 * n:8 * n + 8, 0:64],
                pt[:, :].rearrange("p (h d) -> p h d", d=HD),
            )

        def emit_pv(h):
            hp = (h % 2) * HD
            hm = h // 2
            pTt = pTts.pop(h)
            pv = ps.tile([HD + 1, N512], F32, tag="mm")
            for kt in range(KTS):
                nc.tensor.matmul(
                    pv[:, :],
                    vaug[:, kt, h * 65:(h + 1) * 65],
                    pTt[:, kt, :],
                    start=(kt == 0), stop=(kt == KTS - 1),
                )
            rc = smal.tile([1, N512], F32, tag="rc")
            bc = smal.tile([HD, N512], F32, tag="bc")
            # denom_raw = true_denom + npad (pad cols give exp(0)=1);
            # true_denom >= nvalid * exp(min logit) >> 0, so the fast
            # reciprocal's denorm/zero edge cases cannot occur
            nc.vector.tensor_scalar(
                out=rc[0:1, :], in0=pv[HD:HD + 1, :],
                scalar1=npad_sb, scalar2=None, op0=subtract,
            )
            nc.vector.reciprocal_approx_fast(out=rc[0:1, :], in_=rc[0:1, :])
            nc.gpsimd.partition_broadcast(bc[:, :], rc[0:1, :])
            nc.vector.tensor_mul(xT[hp:hp + HD, hm, :], pv[0:HD, :], bc[:, :])

        # V chunks n=0 first (PV_0..7 read the full n=0 group of vaug),
        # then the PV pipeline with the remaining QKs and V chunks
        # interleaved.
        for i in range(KTS):
            emit_v_chunk(i)
            drain_qk(2)
        for h in range(H):
            emit_pv(h)
            drain_qk(2)
            if h < KTS:
                emit_v_chunk(KTS + h)
                drain_qk(1)
        drain_qk(len(qk_tasks))

        # ---- output projection (dual eviction: bf16 residual + fp8) ----
        wsb = load_w(wp, [nc.gpsimd, nc.sync])
        for m in range(KT):
            pt = ps.tile([P, N512], F32, tag="mm")
            for k in range(KT):
                nc.tensor.matmul(
                    pt[:, :], wsb[:, k, m * P:(m + 1) * P], xT[:, k, :],
                    start=(k == 0), stop=(k == KT - 1),
                )
            nc.vector.tensor_scalar(
                out=xpT[:, m, :], in0=pt[:, :],
                scalar1=bpb2_sb[:, m:m + 1], scalar2=None, op0=add,
            )
            nc.vector.tensor_scalar(
                out=xp8[:, m, :], in0=pt[:, :],
                scalar1=bp_sb[:, m:m + 1], scalar2=SXP, op0=add, op1=mult,
            )

        # ---- MLP in fp8 DoubleRow ----
        wsb = load_w(w1, [nc.gpsimd, nc.sync], dtype=FP8, tag="w8", nchunks=2)
        for m in range(KT):
            pt = ps.tile([P, N512], F32, tag="mm")
            for d in range(KT // 2):
                nc.tensor.matmul(
                    pt[:, :],
                    wsb[:, 2 * d:2 * d + 2, m * P:(m + 1) * P],
                    xp8[:, 2 * d:2 * d + 2, :],
                    start=(d == 0), stop=(d == KT // 2 - 1),
                    perf_mode=DR,
                )
            nc.scalar.activation(
                out=h1T[:, m, :], in_=pt[:, :], func=Act.Gelu,
                bias=b1_sb[:, m:m + 1], scale=INV1,
            )
            nc.vector.tensor_scalar(
                out=h18[:, m, :], in0=h1T[:, m, :],
                scalar1=SH, scalar2=None, op0=mult,
            )

        wsb = load_w(w2, [nc.gpsimd, nc.sync], dtype=FP8, tag="w8", nchunks=2)
        for m in range(KT):
            pt = ps.tile([P, N512], F32, tag="mm")
            for d in range(KT // 2):
                nc.tensor.matmul(
                    pt[:, :],
                    wsb[:, 2 * d:2 * d + 2, m * P:(m + 1) * P],
                    h18[:, 2 * d:2 * d + 2, :],
                    start=(d == 0), stop=(d == KT // 2 - 1),
                    perf_mode=DR,
                )
            ot = outp.tile([P, N512], F32, tag="o")
            nc.vector.scalar_tensor_tensor(
                out=ot[:, :], in0=pt[:, :], scalar=INV2,
                in1=xpT[:, m, :], op0=mult, op1=add,
            )
            nc.sync.dma_start(out[m * P:(m + 1) * P, :], ot[:, :])

    nc.compile()
    return nc


_prog_cache = {}


def _get_program(SV):
    if SV not in _prog_cache:
        _prog_cache[SV] = build_program(SV)
    return _prog_cache[SV]


def make_in_maps(inputs, SV, valid_idx, nvalid):
    q = np.asarray(inputs["query"], np.float32)
    k = np.asarray(inputs["key"], np.float32)
    v = np.asarray(inputs["value"], np.float32)
    Wq = np.asarray(inputs["Wq"], np.float32) * SCALE
    bq = np.asarray(inputs["bq"], np.float32) * SCALE
    Wk = np.asarray(inputs["Wk"], np.float32)
    bk = np.asarray(inputs["bk"], np.float32)
    Wv = np.asarray(inputs["Wv"], np.float32)
    bv = np.asarray(inputs["bv"], np.float32)
    Wp = np.asarray(inputs["Wp"], np.float32)
    bp = np.asarray(inputs["bp"], np.float32)
    W1 = np.asarray(inputs["W1"], np.float32)
    b1 = np.asarray(inputs["b1"], np.float32)
    W2 = np.asarray(inputs["W2"], np.float32)
    b2 = np.asarray(inputs["b2"], np.float32)

    bp_eff = bp + bv @ Wp

    def to_fp8(w):
        return np.ascontiguousarray(
            np.clip(w * SW, -FP8MAX, FP8MAX).astype(NPFP8))

    shared = {
        "wq": np.ascontiguousarray(Wq.astype(NPBF16)),
        "wk": np.ascontiguousarray(Wk.astype(NPBF16)),
        "wv": np.ascontiguousarray(Wv.astype(NPBF16)),
        "wp": np.ascontiguousarray(Wp.astype(NPBF16)),
        "w1": to_fp8(W1),
        "w2": to_fp8(W2),
    }

    def pack_cols(vec):      # [C] -> [P, KT] with [p, j] = vec[j*128+p]
        return np.asarray(vec, np.float32).reshape(KT, P).T

    in_maps = []
    for core in range(NCORES):
        b, qs = divmod(core, 2)
        nv = int(nvalid[b])
        idx = valid_idx[b]
        m = dict(shared)
        m["qt_in"] = np.ascontiguousarray(
            q[b, qs * SQ:(qs + 1) * SQ, :].T.astype(NPBF16))
        ktg = np.zeros((C, SV), NPBF16)
        ktg[:, :nv] = k[b].T[:, idx].astype(NPBF16)
        vtg = np.zeros((C, SV), NPBF16)
        vtg[:, :nv] = v[b].T[:, idx].astype(NPBF16)
        m["kt_in"] = np.ascontiguousarray(ktg)
        m["vt_in"] = np.ascontiguousarray(vtg)
        kmrow = np.zeros((SV,), NPBF16)
        kmrow[:nv] = 1.0
        m["kmask"] = np.ascontiguousarray(np.broadcast_to(kmrow, (P, SV)))
        base = np.zeros((P, 6, KT), np.float32)
        for i, vec in enumerate((bq, bk, bp_eff, b1, bp_eff + b2)):
            base[:, i, :] = pack_cols(vec)
        base[0, 5, 0] = float(SV - nv)
        m["bvecs"] = np.ascontiguousarray(base)
        in_maps.append(m)
    return in_maps


def run(inputs, trace=False, trace_cores=None):
    mask = np.asarray(inputs["mask"])
    combined = (mask[:, :S] != 0) | (mask[:, S:2 * S] != 0)   # [B, S]
    valid_idx = [np.nonzero(combined[b])[0] for b in range(B)]
    nvalid = np.array([len(ix) for ix in valid_idx])
    SV = max(P, int(-(-int(nvalid.max()) // P)) * P)
    nc = _get_program(SV)
    in_maps = make_in_maps(inputs, SV, valid_idx, nvalid)
    res = run_bass_kernel_spmd(
        nc, in_maps, core_ids=list(range(NCORES)),
        trace=trace, trace_cores=trace_cores,
    )
    outfull = np.empty((B, S, C), np.float32)
    for core in range(NCORES):
        b, qs = divmod(core, 2)
        outfull[b, qs * SQ:(qs + 1) * SQ, :] = res.results[core]["out"].T
    return outfull, res


def kernel(**inputs):
    outfull, _ = run(inputs)
    return outfull
